# revision 15
# baseline (speedup 1.0000x reference)
"""DTM (distance-to-measure) kernel for Trainium2, 8 NeuronCores — v3.

Math: F(t) = sum(w*min(d2,t)) + t*(bound - sum(w)) is concave with its max at
the weighted-quantile threshold t*; the reference output is sqrt(F(t*)/bound).
Errors in t enter F only quadratically, so the search can be cheap while one
exact weighted evaluation + a second-order Newton polish give the accuracy.

Measured op economics on this silicon (REPS-slope microbench): full-row fp16
tensor_scalar with an IMMEDIATE scalar + accum runs ~346ns for [128,2048]
(fast perf mode); AP-scalar tensor_scalar (~3185ns), sliced rows (~1499ns for
512 cols!) and fp16 STT (~3860ns) are all slow. fp32 STT with AP scalar is
~1725ns. Hence:
  1. ACT normalizes each tile's d2 with per-tile scale/bias APs (free on ACT):
     d2n = (d2 - lo_t)/(hi_t - lo_t), where [lo_t, hi_t] comes from a cheap
     host-side subsampled quantile estimate. A GLOBAL immediate threshold grid
     j/J then scans all cores/tiles: J=8 full-row imm fp16 counts per tile.
  2. Bucket selection, t_hat extraction and the Newton polish
     F += wbar*(0.3M - C)^2 * dt/(2*(C - Cprev)) are batched [P,NT(,J)]
     tensor_tensor algebra (no AP-scalar tensor_scalar ops anywhere).
  3. One exact fp32 STT final per tile: accum(min(d2,t_hat)*w).
All DVE writes are >=2 ops old before any read (measured stale-read hazard);
the 13 STT finals are interleaved into the algebra chain as natural spacers.
"""
import sys
sys.path.insert(0, "/opt/trn_rl_repo")

import numpy as np
import concourse.bass as bass
from concourse import mybir

M0 = 0.3
B, M, N = 2, 2048, 6561
P = 128
NT = 13              # tiles per core
NSH = NT * P         # 1664 grid points per core
NSHARDS = 4
NPAD = NSH * NSHARDS
CHUNK = 512
NCH = M // CHUNK
J = 8                # scan thresholds per tile (immediates j/J on d2n)
MSUB = 512           # host subsample for per-tile quantile ranges
BND = 0.3 * M        # count-space bound (batch-independent)
REPS = 1             # bench amplifier: DVE program repeated REPS times

_NC = None


def _build():
    global _NC
    if _NC is not None:
        return _NC
    nc = bass.Bass()
    f32 = mybir.dt.float32
    f16 = mybir.dt.float16

    gaug = nc.dram_tensor("gaug", [4, NSH], f32, kind="ExternalInput")
    xaug = nc.dram_tensor("xaug", [4, M], f32, kind="ExternalInput")
    w32r = nc.dram_tensor("w32r", [1, M], f32, kind="ExternalInput")
    losc = nc.dram_tensor("losc", [1, 2, NT], f32, kind="ExternalInput")
    rows = nc.dram_tensor("rows", [1, 5, NT], f32, kind="ExternalInput")
    out = nc.dram_tensor("out", [P, NT], f32, kind="ExternalOutput")

    sb_gaug = nc.alloc_sbuf_tensor("sb_gaug", [4, NSH], f32)
    sb_xaug = nc.alloc_sbuf_tensor("sb_xaug", [4, M], f32)
    sb_w32 = nc.alloc_sbuf_tensor("sb_w32", [P, M], f32)
    sb_losc = nc.alloc_sbuf_tensor("sb_losc", [P, 2, NT], f32)
    sb_rows = nc.alloc_sbuf_tensor("sb_rows", [P, 5, NT], f32)
    sb_out = nc.alloc_sbuf_tensor("sb_out", [P, NT], f32)
    d2h = [nc.alloc_sbuf_tensor(f"d2h_{t}", [P, M], f16) for t in range(NT)]
    d2f = [nc.alloc_sbuf_tensor(f"d2f_{t}", [P, M], f32) for t in range(NT)]
    scr16 = nc.alloc_sbuf_tensor("scr16", [P, M], f16)
    scr32 = nc.alloc_sbuf_tensor("scr32", [P, M], f32)
    cntS = nc.alloc_sbuf_tensor("cntS", [P, NT, J], f32)
    mm_ = nc.alloc_sbuf_tensor("mm_", [P, NT, J], f32)
    doh = nc.alloc_sbuf_tensor("doh", [P, NT, J - 1], f32)
    CS1 = nc.alloc_sbuf_tensor("CS1", [P, NT, J - 1], f32)
    CS0 = nc.alloc_sbuf_tensor("CS0", [P, NT, J - 1], f32)
    SELt = nc.alloc_sbuf_tensor("SELt", [P, NT], f32)
    tn = nc.alloc_sbuf_tensor("tn", [P, NT], f32)
    oh0 = nc.alloc_sbuf_tensor("oh0", [P, NT], f32)
    q1 = nc.alloc_sbuf_tensor("q1", [P, NT], f32)
    tbF = nc.alloc_sbuf_tensor("tbF", [P, NT], f32)
    CselA = nc.alloc_sbuf_tensor("CselA", [P, NT], f32)
    CselB = nc.alloc_sbuf_tensor("CselB", [P, NT], f32)
    Csel = nc.alloc_sbuf_tensor("Csel", [P, NT], f32)
    Cprev = nc.alloc_sbuf_tensor("Cprev", [P, NT], f32)
    den0 = nc.alloc_sbuf_tensor("den0", [P, NT], f32)
    den = nc.alloc_sbuf_tensor("den", [P, NT], f32)
    resc = nc.alloc_sbuf_tensor("resc", [P, NT], f32)
    r2 = nc.alloc_sbuf_tensor("r2", [P, NT], f32)
    n1 = nc.alloc_sbuf_tensor("n1", [P, NT], f32)
    n2 = nc.alloc_sbuf_tensor("n2", [P, NT], f32)
    qq0 = nc.alloc_sbuf_tensor("qq0", [P, NT], f32)
    rec = nc.alloc_sbuf_tensor("rec", [P, NT], f32)
    qq = nc.alloc_sbuf_tensor("qq", [P, NT], f32)
    vv = nc.alloc_sbuf_tensor("vv", [P, NT], f32)
    o1 = nc.alloc_sbuf_tensor("o1", [P, NT], f32)
    msumT = nc.alloc_sbuf_tensor("msumT", [P, NT], f32)
    spb = nc.alloc_sbuf_tensor("spb", [P, 1], f32)
    ps = [nc.alloc_psum_tensor(f"ps_{i}", [P, M], f32) for i in range(2)]

    Alu = mybir.AluOpType
    Act = mybir.ActivationFunctionType

    with (
        nc.Block() as block,
        nc.semaphore("dma_sem") as dma_sem,
        nc.semaphore("mm_sem") as mm_sem,
        nc.semaphore("d2_sem") as d2_sem,
        nc.semaphore("done_sem") as done_sem,
    ):
        @block.sync
        def _(sync):
            sync.dma_start(out=sb_gaug[:], in_=gaug[:, :]).then_inc(dma_sem, 16)
            sync.dma_start(out=sb_xaug[:], in_=xaug[:, :]).then_inc(dma_sem, 16)
            sync.dma_start(out=sb_losc[:], in_=losc[:, :, :].to_broadcast((P, 2, NT))).then_inc(dma_sem, 16)
            sync.dma_start(out=sb_w32[:], in_=w32r[:, :].to_broadcast((P, M))).then_inc(dma_sem, 16)
            sync.dma_start(out=sb_rows[:], in_=rows[:, :, :].to_broadcast((P, 5, NT))).then_inc(dma_sem, 16)

        @block.tensor
        def _(tensor):
            tensor.wait_ge(dma_sem, 32)
            for t in range(NT):
                if t >= 2:
                    tensor.wait_ge(d2_sem, t - 1)
                mm = None
                for c in range(NCH):
                    mm = tensor.matmul(
                        out=ps[t % 2][:, c * CHUNK:(c + 1) * CHUNK],
                        lhsT=sb_gaug[:, t * P:(t + 1) * P],
                        rhs=sb_xaug[:, c * CHUNK:(c + 1) * CHUNK],
                        start=True, stop=True)
                mm.then_inc(mm_sem, 1)

        @block.scalar
        def _(scalar):
            scalar.wait_ge(dma_sem, 48)
            for t in range(NT):
                scalar.wait_ge(mm_sem, t + 1)
                # normalized fp16 copy for the imm scans
                scalar.activation(out=d2h[t][:], in_=ps[t % 2][:],
                                  func=Act.Relu,
                                  scale=sb_losc[:, 0, t:t + 1],
                                  bias=sb_losc[:, 1, t:t + 1])
                # raw fp32 copy for the exact weighted final
                scalar.activation(out=d2f[t][:], in_=ps[t % 2][:],
                                  func=Act.Relu).then_inc(d2_sem, 1)

        @block.vector
        def _(vector):
            vector.wait_ge(dma_sem, 80)
            aRow = sb_rows[:, 0, :]
            bRow = sb_rows[:, 1, :]
            dRow = sb_rows[:, 2, :]
            WMrow = sb_rows[:, 3, :]
            B2row = sb_rows[:, 4, :]

            def sp():
                vector.tensor_scalar(out=spb[:], in0=sb_rows[:, 0, 0:1],
                                     scalar1=0.0, scalar2=None, op0=Alu.mult)

            def fin(t):
                return vector.scalar_tensor_tensor(
                    out=scr32[:], in0=d2f[t][:], scalar=tbF[:, t:t + 1],
                    in1=sb_w32[:], op0=Alu.min, op1=Alu.mult,
                    accum_out=msumT[:, t:t + 1])

            last = None
            for rep in range(REPS):
                for t in range(NT):
                    if rep == 0:
                        vector.wait_ge(d2_sem, t + 1)
                    for j in range(J):
                        vector.tensor_scalar(
                            out=scr16[:], in0=d2h[t][:],
                            scalar1=(j + 1) / J, scalar2=0.0, op0=Alu.is_lt,
                            op1=Alu.add, accum_out=cntS[:, t, j:j + 1])
                # selection + polish algebra, finals interleaved as spacers;
                # every DVE write is >=2 ops old at each read
                TT = vector.tensor_tensor
                TS = vector.tensor_scalar
                sp()
                TS(out=mm_[:, :, :], in0=cntS[:, :, :], scalar1=BND,
                   scalar2=None, op0=Alu.is_lt)
                sp()
                TT(out=doh[:, :, :], in0=mm_[:, :, 0:J - 1],
                   in1=mm_[:, :, 1:J], op=Alu.subtract)
                vector.tensor_reduce(out=SELt[:, :], in_=mm_[:, :, :],
                                     axis=mybir.AxisListType.X, op=Alu.add)
                TT(out=CS1[:, :, :], in0=doh[:, :, :], in1=cntS[:, :, 1:J],
                   op=Alu.mult)
                TT(out=CS0[:, :, :], in0=doh[:, :, :], in1=cntS[:, :, 0:J - 1],
                   op=Alu.mult)
                TS(out=tn[:, :], in0=SELt[:, :], scalar1=1.0 / J,
                   scalar2=1.0 / J, op0=Alu.mult, op1=Alu.add)
                vector.tensor_reduce(out=CselA[:, :], in_=CS1[:, :, :],
                                     axis=mybir.AxisListType.X, op=Alu.add)
                vector.tensor_reduce(out=Cprev[:, :], in_=CS0[:, :, :],
                                     axis=mybir.AxisListType.X, op=Alu.add)
                TS(out=oh0[:, :], in0=mm_[:, :, 0], scalar1=-1.0, scalar2=1.0,
                   op0=Alu.mult, op1=Alu.add)
                TT(out=q1[:, :], in0=tn[:, :], in1=aRow, op=Alu.mult)
                TT(out=CselB[:, :], in0=oh0[:, :], in1=cntS[:, :, 0],
                   op=Alu.mult)
                TT(out=tbF[:, :], in0=q1[:, :], in1=bRow, op=Alu.add)
                TT(out=Csel[:, :], in0=CselA[:, :], in1=CselB[:, :],
                   op=Alu.add)
                last = fin(0)
                TT(out=den0[:, :], in0=Csel[:, :], in1=Cprev[:, :],
                   op=Alu.subtract)
                last = fin(1)
                TS(out=resc[:, :], in0=Csel[:, :], scalar1=BND, scalar2=None,
                   op0=Alu.subtract)
                last = fin(2)
                TS(out=den[:, :], in0=den0[:, :], scalar1=1.0, scalar2=None,
                   op0=Alu.max)
                last = fin(3)
                TT(out=r2[:, :], in0=resc[:, :], in1=resc[:, :], op=Alu.mult)
                last = fin(4)
                TT(out=n1[:, :], in0=r2[:, :], in1=dRow, op=Alu.mult)
                last = fin(5)
                TS(out=n2[:, :], in0=n1[:, :], scalar1=0.5, scalar2=None,
                   op0=Alu.mult)
                last = fin(6)
                vector.reciprocal(out=rec[:, :], in_=den[:, :])
                last = fin(7)
                TT(out=qq0[:, :], in0=n2[:, :], in1=rec[:, :], op=Alu.mult)
                last = fin(8)
                TT(out=qq[:, :], in0=qq0[:, :], in1=WMrow, op=Alu.mult)
                last = fin(9)
                TT(out=vv[:, :], in0=tbF[:, :], in1=B2row, op=Alu.mult)
                last = fin(10)
                TT(out=o1[:, :], in0=vv[:, :], in1=qq[:, :], op=Alu.add)
                last = fin(11)
                last = fin(12)
                sp()
                last = vector.tensor_tensor(out=sb_out[:, :], in0=o1[:, :],
                                            in1=msumT[:, :], op=Alu.add)
                last.then_inc(done_sem, 1)

        @block.sync
        def _(sync):
            sync.wait_ge(done_sem, REPS)
            sync.dma_start(out=out[:, :], in_=sb_out[:]).then_inc(dma_sem, 16)
            sync.wait_ge(dma_sem, 96)

    _NC = nc
    return nc


def _prepare_in_maps(inputs, weight, grid):
    inputs = np.asarray(inputs, dtype=np.float32)
    weight = np.asarray(weight, dtype=np.float32)
    grid = np.asarray(grid, dtype=np.float32)

    gpad = np.zeros((NPAD, 2), dtype=np.float32)
    gpad[:N] = grid
    G2 = (gpad * gpad).sum(-1)
    gaug_full = np.stack([-2.0 * gpad[:, 0], -2.0 * gpad[:, 1], G2,
                          np.ones(NPAD, np.float32)], 0).astype(np.float32)

    in_maps = []
    wB = np.empty(B, np.float32)
    per_batch = []
    for b in range(B):
        X = inputs[b]
        X2 = (X * X).sum(-1)
        xaug_np = np.stack([X[:, 0], X[:, 1], np.ones(M, np.float32), X2],
                           0).astype(np.float32)
        w = weight[b]
        sw = w.sum(dtype=np.float32)
        wB[b] = M0 * sw
        per_batch.append((xaug_np, w[None, :].astype(np.float32),
                          np.float32(sw / M), np.float32(wB[b] - sw)))
    sub = inputs[:, :MSUB, :]           # [B, MSUB, 2] host quantile subsample
    for c in range(8):
        b = c // NSHARDS
        s = c % NSHARDS
        xaug_np, w32row, wbar, bnd2 = per_batch[b]
        gs = gpad[s * NSH:(s + 1) * NSH]
        # per-tile threshold ranges from subsampled 0.3-quantiles
        d2s = ((gs[:, None, :] - sub[b][None, :, :]) ** 2).sum(-1)
        q = np.quantile(d2s, M0, axis=1).reshape(NT, P)
        lo = np.maximum(q.min(1) * 0.85 - 0.04, 1e-3).astype(np.float32)
        hi = (q.max(1) * 1.15 + 0.04).astype(np.float32)
        a = hi - lo                      # = J * delta
        sc = (1.0 / a).astype(np.float32)
        bias = (-lo * sc).astype(np.float32)
        losc_np = np.stack([sc, bias], 0)[None].astype(np.float32)
        rows_np = np.stack([a, lo, a / J,
                            np.full(NT, wbar, np.float32),
                            np.full(NT, bnd2, np.float32)], 0)[None]
        in_maps.append({
            "gaug": np.ascontiguousarray(gaug_full[:, s * NSH:(s + 1) * NSH]),
            "xaug": xaug_np,
            "w32r": w32row,
            "losc": np.ascontiguousarray(losc_np),
            "rows": np.ascontiguousarray(rows_np.astype(np.float32)),
        })
    return in_maps, wB


def _gather(results, wB):
    sel = np.empty((B, NPAD), np.float32)
    for c in range(8):
        b = c // NSHARDS
        s = c % NSHARDS
        vals = results[c]["out"]            # [P, NT]; grid idx = t*P + p
        sel[b, s * NSH:(s + 1) * NSH] = vals.T.reshape(-1)
    sel = sel[:, :N]
    out = np.sqrt(np.maximum(sel, 0.0) / wB[:, None]).astype(np.float32)
    return out


def _make_runner(nc, n_cores=8):
    """Compile once; return a reusable sharded callable."""
    import jax
    from jax.sharding import Mesh, PartitionSpec
    from jax.experimental.shard_map import shard_map
    from concourse import bass2jax
    import concourse.mybir as _mybir

    bass2jax.install_neuronx_cc_hook()
    in_names, out_names, out_avals = [], [], []
    for alloc in nc.m.functions[0].allocations:
        if not isinstance(alloc, _mybir.MemoryLocationSet):
            continue
        name = alloc.memorylocations[0].name
        if alloc.kind == "ExternalInput":
            if not (nc.partition_id_tensor is not None
                    and name == nc.partition_id_tensor.name):
                in_names.append(name)
        elif alloc.kind == "ExternalOutput":
            out_names.append(name)
            out_avals.append(jax.core.ShapedArray(
                tuple(alloc.tensor_shape), _mybir.dt.np(alloc.dtype)))
    n_params = len(in_names)
    all_names = list(in_names) + list(out_names)
    has_pid = nc.partition_id_tensor is not None
    if has_pid:
        all_names.append(nc.partition_id_tensor.name)

    def _body(*args):
        operands = list(args)
        if has_pid:
            operands.append(bass2jax.partition_id_tensor())
        outs = bass2jax._bass_exec_p.bind(
            *operands, out_avals=tuple(out_avals), in_names=tuple(all_names),
            out_names=tuple(out_names), lowering_input_output_aliases=(),
            sim_require_finite=True, sim_require_nnan=True, nc=nc)
        return tuple(outs)

    devices = jax.devices()[:n_cores]
    mesh = Mesh(np.asarray(devices), ("core",))
    nio = n_params + len(out_names)
    sharded = jax.jit(
        shard_map(_body, mesh=mesh, in_specs=(PartitionSpec("core"),) * nio,
                  out_specs=(PartitionSpec("core"),) * len(out_names),
                  check_rep=False),
        keep_unused=True)

    def run(in_maps):
        per_core = [[np.asarray(m[name]) for name in in_names] for m in in_maps]
        concat_in = [np.concatenate([per_core[c][i] for c in range(n_cores)], 0)
                     for i in range(n_params)]
        concat_zeros = [np.zeros((n_cores * a.shape[0], *a.shape[1:]), a.dtype)
                        for a in out_avals]
        outs = sharded(*concat_in, *concat_zeros)
        outs = [np.asarray(o) for o in outs]
        return [{name: outs[i].reshape(n_cores, *out_avals[i].shape)[c]
                 for i, name in enumerate(out_names)} for c in range(n_cores)]

    return run


_RUNNER = None


def _get_runner():
    global _RUNNER
    if _RUNNER is None:
        _RUNNER = _make_runner(_build())
    return _RUNNER


def kernel(inputs, weight, grid):
    in_maps, wB = _prepare_in_maps(inputs, weight, grid)
    global _RUNNER
    try:
        results = _get_runner()(in_maps)
    except Exception:
        _RUNNER = None
        results = _get_runner()(in_maps)
    return _gather(results, wB)
